# revision 32
# baseline (speedup 1.0000x reference)
"""Trainium2 Bass kernel for nn_DecompMultiTransform (RGCN basis-decomposition).

Reference computation:
    full_w = (w_comp @ weight).reshape(64, 256, 256)   # per-type weights
    out[n, :] = x[n, :] @ full_w[xtype[n]]             # N = 4096

Scheme (type-parallel, minimal FLOPs):
  Host sorts rows by type into 64 zero-padded groups of CAP=128 rows (pure
  layout - permutation, padding, transpose, bf16 cast). Core c owns types
  8c..8c+7 (<= 1024 row-slots per core). On device:

  Stage 1 - build this core's 8 per-type weight matrices on the PE:
      W_tau[i, o] = sum_b w_comp[tau, b] * weight[b, i*256+o]
    The contraction K packs (r=8 o-columns x b=16 bases) = 128 so the PE
    runs full-K matmuls:  lhsT = wstack_g[(r,b), j]  (a host re-layout of
    weight), rhs = cdelta[(r,b), (half, r', t)] which holds w_comp values
    delta-masked on r==r' (hi/lo bf16 split of the f32 value, so w_comp
    enters exactly). 64 matmuls of [K=128, M=128, N=128] produce
    W_tau[i, o] tiles with i on partitions; a strided add (hi+lo) moves
    them PSUM->SBUF as bf16.

  Stage 2 - per type: out_t[n, :] = x_t[n, :] @ W_tau with x stationary:
      lhsT = xsT[i, n] (128 rows), rhs = W_tau[i, :] moving. 16 matmuls
    of [K=128, M=128, N=256], PSUM-accumulated over the two i-halves.

  All matmul operands are bf16 (PSUM accumulates f32): halves HBM traffic
  and doubles PE row rate vs f32r. Host un-sorts the output.
"""

import sys

if "/opt/trn_rl_repo" not in sys.path:
    sys.path.insert(0, "/opt/trn_rl_repo")

import numpy as np

import concourse.bass as bass
import concourse.mybir as mybir
import concourse.tile as tile
from concourse import bacc
from concourse.bass_utils import run_bass_kernel_spmd

P = 128
N_FULL = 4096
IN_DIM = 256
OUT_DIM = 256
NUM_B = 16
NUM_T = 64
N_CORES = 8
TPC = NUM_T // N_CORES            # 8 types per core
CAP = 128                         # padded rows per type
NG = 64                           # stage-1 groups: (ih 2) x (og 32)
N_CHUNKS = 8                      # weight DMA chunks (8 groups each)
G_PER_BANK = 8                    # stage-1 psum tiles hold 8 groups (1 bank)

F32 = mybir.dt.float32
BF16 = mybir.dt.bfloat16
NP_BF16 = mybir.dt.np(BF16)


def _build_program():
    nc = bacc.Bacc("TRN2", target_bir_lowering=False, debug=False)

    xsT = nc.declare_dram_parameter("xsT", [P, 2, TPC, CAP], BF16, isOutput=False)
    cdelta = nc.declare_dram_parameter("cdelta", [P, 8 * TPC], BF16, isOutput=False)
    wstack = nc.declare_dram_parameter("wstack", [P, NG, P], BF16, isOutput=False)
    outb = nc.declare_dram_parameter("outb", [TPC, CAP, OUT_DIM], BF16, isOutput=True)

    with tile.TileContext(nc) as tc:
        with (
            tc.tile_pool(name="const", bufs=1) as constp,
            tc.tile_pool(name="wpool", bufs=1) as wpool,
            tc.tile_pool(name="wsbp", bufs=1) as wsbp,
            tc.tile_pool(name="stp", bufs=8) as stp,
            tc.tile_pool(name="ps1", bufs=3, space="PSUM") as ps1,
            tc.tile_pool(name="pso", bufs=1, space="PSUM") as pso,
        ):
            # ---- input DMAs. Few, large transfers: one InstDMACopy already
            # fans out across the ring's 16 SDMA engines, and each DMA's
            # completion semaphore lands ~2.5us after its data, so per-chunk
            # semaphores are expensive. 4 weight chunks of 16 groups,
            # alternating rings; cd+xsT lead the scalar ring.
            cd = constp.tile([P, 8 * TPC], BF16, name="cd")
            nc.scalar.dma_start(out=cd[:], in_=cdelta.ap()[:, :])
            xst = constp.tile([P, 2, TPC, CAP], BF16, name="xst")

            # weight chunks (groups): A16 B16 C16 C2-8 D8. The last two
            # banks ride as small late chunks so the endgame chain
            # (last bank -> combine -> stage 2 -> drain) starts early.
            # sync: [A, C, D]; scalar: [cd, B, xsT(t0-3), C2, xsT(t4-7)].
            wchunks = []
            for g0, sz in ((0, 16), (16, 16), (32, 16), (48, 8), (56, 8)):
                wt = wpool.tile([P, sz, P], BF16, name=f"w{g0}")
                wchunks.append((g0, sz, wt))

            def wdma(ring, ci):
                g0, sz, wt = wchunks[ci]
                ring.dma_start(out=wt[:], in_=wstack.ap()[:, g0 : g0 + sz, :])

            wdma(nc.sync, 0)       # A: banks 0-1
            wdma(nc.scalar, 1)     # B: banks 2-3
            wdma(nc.sync, 2)       # C: banks 4-5
            nc.scalar.dma_start(
                out=xst[:, :, 0 : TPC // 2, :], in_=xsT.ap()[:, :, 0 : TPC // 2, :]
            )
            wdma(nc.sync, 4)       # D: bank 7
            wdma(nc.scalar, 3)     # C2: bank 6
            nc.scalar.dma_start(
                out=xst[:, :, TPC // 2 :, :], in_=xsT.ap()[:, :, TPC // 2 :, :]
            )

            def wslice(s):
                for gs, sz, wt in wchunks:
                    if gs <= s < gs + sz:
                        return wt[:, s - gs, :]
                raise AssertionError(s)

            wsb = [
                wsbp.tile([P, TPC, OUT_DIM], BF16, name=f"wsb{ih}") for ih in range(2)
            ]

            # ---- stage 1 / stage 2 interleaved ----
            # Step order covers banks (ih0,og0-7),(ih1,og0-7),(ih0,og8-15),
            # (ih1,og8-15),... so after 4 banks the o-half 0 of every W_tau
            # is complete and stage 2's oh=0 matmuls can fill DMA-wait gaps.
            pos = [
                pso.tile([P, 2, OUT_DIM], F32, name=f"po{i}", space="PSUM")
                for i in range(TPC // 2)
            ]

            def s2_mm(t):
                po = pos[t // 2]
                for ih in range(2):
                    nc.tensor.matmul(
                        out=po[:, t % 2, :],
                        lhsT=xst[:, ih, t, :],
                        rhs=wsb[ih][:, t, :],
                        start=(ih == 0),
                        stop=(ih == 1),
                    )

            def drain_pair(p):
                po = pos[p]
                st = stp.tile([P, 2, OUT_DIM], BF16, name="st", tag="st")
                if p % 2 == 0:
                    nc.vector.tensor_copy(out=st[:], in_=po[:])
                else:
                    nc.scalar.copy(st[:], po[:])
                deng = nc.sync if p % 2 == 0 else nc.scalar
                deng.dma_start(
                    out=outb.ap()[2 * p : 2 * p + 2].rearrange("t p o -> p t o"),
                    in_=st[:],
                )

            for s in range(NG):
                b, k = divmod(s, G_PER_BANK)
                if k == 0:
                    ps = ps1.tile(
                        [P, G_PER_BANK, 8 * TPC], F32, name="ps1", tag="ps1",
                        space="PSUM",
                    )
                nc.tensor.matmul(
                    out=ps[:, k, :],
                    lhsT=wslice(s),
                    rhs=cd[:],
                    start=True,
                    stop=True,
                )
                if k == G_PER_BANK - 1:
                    # scatter-copy the bank into W_sb[ih][:, t, o] (bf16)
                    ih, og0 = b % 2, (b // 2) * 8
                    src = ps[:].rearrange(
                        "p gl (rp t) -> p t gl rp", rp=8, t=TPC
                    )
                    dst = wsb[ih][:][:, :, og0 * 8 : (og0 + 8) * 8].rearrange(
                        "p t (gl rp) -> p t gl rp", gl=G_PER_BANK, rp=8
                    )
                    if b % 2 == 0:
                        nc.vector.tensor_copy(out=dst, in_=src)
                    else:
                        nc.scalar.copy(dst, src)

            # stage 2; drain each type pair as it completes
            for t in range(TPC):
                s2_mm(t)
                if t % 2 == 1:
                    drain_pair(t // 2)

    nc.compile()
    return nc


_PROGRAM = None
LAST_RESULT = None  # test harness introspection


def kernel(x, xtype, weight, w_comp, trace=False):
    global _PROGRAM, LAST_RESULT
    x = np.asarray(x, dtype=np.float32)
    xtype = np.asarray(xtype).astype(np.int64)
    weight = np.asarray(weight, dtype=np.float32)
    w_comp = np.asarray(w_comp, dtype=np.float32)
    assert x.shape == (N_FULL, IN_DIM) and weight.shape == (NUM_B, IN_DIM * OUT_DIM)

    if _PROGRAM is None:
        _PROGRAM = _build_program()
    nc = _PROGRAM

    # ---- host-side layout: sort rows by type into padded slots ----
    counts = np.bincount(xtype, minlength=NUM_T)
    if counts.max() > CAP:
        raise RuntimeError(f"type count {counts.max()} exceeds CAP={CAP}")
    order = np.argsort(xtype, kind="stable")
    sorted_t = xtype[order]
    starts = np.zeros(NUM_T, dtype=np.int64)
    starts[1:] = np.cumsum(counts)[:-1]
    rank = np.arange(N_FULL, dtype=np.int64) - starts[sorted_t]
    slot = sorted_t * CAP + rank  # global padded slot per sorted row

    xpad = np.zeros((NUM_T * CAP, IN_DIM), np.float32)
    xpad[slot] = x[order]
    xpad = xpad.astype(NP_BF16)

    # wstack[(r,b), g, j] = weight[b, (ih*128+j)*256 + og*8+r], g = ih*32+og,
    # with columns permuted into the device's step order (banks alternate
    # ih so each o-half of W completes as early as possible).
    w5 = weight.reshape(NUM_B, 2, P, 32, 8)  # b, ih, j, og, r
    wst_nat = np.ascontiguousarray(w5.transpose(4, 0, 1, 3, 2)).reshape(P, NG, P)
    perm = np.empty(NG, np.int64)
    for s in range(NG):
        b, k = divmod(s, G_PER_BANK)
        ih, og = b % 2, (b // 2) * 8 + k
        perm[s] = ih * 32 + og
    wstack = np.ascontiguousarray(wst_nat[:, perm, :]).astype(NP_BF16)

    c_bf = w_comp.astype(NP_BF16)

    in_maps = []
    for c in range(N_CORES):
        xc = xpad[c * TPC * CAP : (c + 1) * TPC * CAP]  # [1024, 256] bf16
        xsT = np.ascontiguousarray(
            xc.reshape(TPC, CAP, 2, P).transpose(3, 2, 0, 1)
        )  # [i, ih, t, n]
        cdl = np.zeros((8, NUM_B, 8, TPC), NP_BF16)  # r, b, rp, t
        for r in range(8):
            cdl[r, :, r, :] = c_bf[c * TPC : (c + 1) * TPC, :].T
        in_maps.append(
            {
                "xsT": xsT,
                "cdelta": cdl.reshape(P, 8 * TPC),
                "wstack": wstack,
            }
        )

    res = run_bass_kernel_spmd(nc, in_maps, list(range(N_CORES)), trace=trace)
    LAST_RESULT = res

    big = np.stack([res.results[c]["outb"] for c in range(N_CORES)]).reshape(
        NUM_T * CAP, OUT_DIM
    )
    out = np.empty((N_FULL, OUT_DIM), np.float32)
    out[order] = big[slot].astype(np.float32)
    return out


# revision 36
# speedup vs baseline: 1.0978x; 1.0978x over previous
"""Trainium2 Bass kernel for nn_DecompMultiTransform (RGCN basis-decomposition).

Reference computation:
    full_w = (w_comp @ weight).reshape(64, 256, 256)   # per-type weights
    out[n, :] = x[n, :] @ full_w[xtype[n]]             # N = 4096

Scheme (type-parallel, minimal FLOPs):
  Host sorts rows by type into 64 zero-padded groups of CAP=128 rows (pure
  layout - permutation, padding, transpose, bf16 cast). Core c owns types
  8c..8c+7 (<= 1024 row-slots per core). On device:

  Stage 1 - build this core's 8 per-type weight matrices on the PE:
      W_tau[i, o] = sum_b w_comp[tau, b] * weight[b, i*256+o]
    The contraction K packs (r=8 o-columns x b=16 bases) = 128 so the PE
    runs full-K matmuls:  lhsT = wstack_g[(r,b), j]  (a host re-layout of
    weight), rhs = cdelta[(r,b), (r', t)] which holds w_comp values
    delta-masked on r==r'. 64 matmuls of [K=128, M=128, N=64] produce
    W_tau[i, o] tiles with i on partitions; strided copies move them
    PSUM->SBUF as bf16.

  Stage 2 - per type: out_t[n, :] = x_t[n, :] @ W_tau with x stationary:
      lhsT = xsT[i, n] (128 rows), rhs = W_tau[i, :] moving. 16 matmuls
    of [K=128, M=128, N=256], PSUM-accumulated over the two i-halves.

  All matmul operands are bf16 (PSUM accumulates f32): halves HBM traffic
  and doubles PE row rate vs f32r. Host un-sorts the output.
"""

import sys

if "/opt/trn_rl_repo" not in sys.path:
    sys.path.insert(0, "/opt/trn_rl_repo")

import numpy as np

import concourse.bass as bass
import concourse.mybir as mybir
import concourse.tile as tile
from concourse import bacc
from concourse.bass_utils import run_bass_kernel_spmd

P = 128
N_FULL = 4096
IN_DIM = 256
OUT_DIM = 256
NUM_B = 16
NUM_T = 64
N_CORES = 8
TPC = NUM_T // N_CORES            # 8 types per core
CAP = 128                         # padded rows per type
NG = 64                           # stage-1 groups: (ih 2) x (og 32)
N_CHUNKS = 8                      # weight DMA chunks (8 groups each)
G_PER_BANK = 8                    # stage-1 psum tiles hold 8 groups (1 bank)

F32 = mybir.dt.float32
BF16 = mybir.dt.bfloat16
NP_BF16 = mybir.dt.np(BF16)


def _build_program():
    nc = bacc.Bacc("TRN2", target_bir_lowering=False, debug=False)

    xsT = nc.declare_dram_parameter("xsT", [P, 2, TPC, CAP], BF16, isOutput=False)
    cdelta = nc.declare_dram_parameter("cdelta", [P, 8 * TPC], BF16, isOutput=False)
    wstack = nc.declare_dram_parameter("wstack", [P, NG, P], BF16, isOutput=False)
    outb = nc.declare_dram_parameter("outb", [TPC, CAP, OUT_DIM], BF16, isOutput=True)

    with tile.TileContext(nc) as tc:
        with (
            tc.tile_pool(name="const", bufs=1) as constp,
            tc.tile_pool(name="wpool", bufs=1) as wpool,
            tc.tile_pool(name="wsbp", bufs=1) as wsbp,
            tc.tile_pool(name="stp", bufs=8) as stp,
            tc.tile_pool(name="ps1", bufs=3, space="PSUM") as ps1,
            tc.tile_pool(name="pso", bufs=1, space="PSUM") as pso,
        ):
            # ---- input DMAs. Few, large transfers: one InstDMACopy already
            # fans out across the ring's 16 SDMA engines, and each DMA's
            # completion semaphore lands ~2.5us after its data, so per-chunk
            # semaphores are expensive. 4 weight chunks of 16 groups,
            # alternating rings; cd+xsT lead the scalar ring.
            cd = constp.tile([P, 8 * TPC], BF16, name="cd")
            nc.scalar.dma_start(out=cd[:], in_=cdelta.ap()[:, :])
            xst = constp.tile([P, 2, TPC, CAP], BF16, name="xst")

            # weight chunks (groups): A16 B16 C16 C2-8 D8. The last two
            # banks ride as small late chunks so the endgame chain
            # (last bank -> combine -> stage 2 -> drain) starts early.
            # sync: [A, C, D]; scalar: [cd, B, xsT(t0-3), C2, xsT(t4-7)].
            wchunks = []
            for g0, sz in ((0, 16), (16, 16), (32, 16), (48, 8), (56, 8)):
                wt = wpool.tile([P, sz, P], BF16, name=f"w{g0}")
                wchunks.append((g0, sz, wt))

            def wdma(ring, ci):
                g0, sz, wt = wchunks[ci]
                ring.dma_start(out=wt[:], in_=wstack.ap()[:, g0 : g0 + sz, :])

            wdma(nc.sync, 0)       # A: banks 0-1
            wdma(nc.scalar, 1)     # B: banks 2-3
            wdma(nc.sync, 2)       # C: banks 4-5
            nc.scalar.dma_start(
                out=xst[:, :, 0 : TPC // 2, :], in_=xsT.ap()[:, :, 0 : TPC // 2, :]
            )
            wdma(nc.sync, 4)       # D: bank 7
            wdma(nc.scalar, 3)     # C2: bank 6
            nc.scalar.dma_start(
                out=xst[:, :, TPC // 2 :, :], in_=xsT.ap()[:, :, TPC // 2 :, :]
            )

            def wslice(s):
                for gs, sz, wt in wchunks:
                    if gs <= s < gs + sz:
                        return wt[:, s - gs, :]
                raise AssertionError(s)

            wsb = [
                wsbp.tile([P, TPC, OUT_DIM], BF16, name=f"wsb{ih}") for ih in range(2)
            ]

            # ---- stage 1 then stage 2 ----
            # Step order covers banks (ih0,og0-7),(ih1,og0-7),(ih0,og8-15),
            # (ih1,og8-15),...; each bank's PSUM is scatter-copied into
            # W_sb right after its 8 matmuls so copies overlap the PE.
            pos = [
                pso.tile([P, 2, OUT_DIM], F32, name=f"po{i}", space="PSUM")
                for i in range(TPC // 2)
            ]

            def s2_mm(t):
                po = pos[t // 2]
                for ih in range(2):
                    nc.tensor.matmul(
                        out=po[:, t % 2, :],
                        lhsT=xst[:, ih, t, :],
                        rhs=wsb[ih][:, t, :],
                        start=(ih == 0),
                        stop=(ih == 1),
                    )

            def drain(t):
                po = pos[t // 2]
                st = stp.tile([P, OUT_DIM], BF16, name="st", tag="st")
                if t % 2 == 0:
                    nc.scalar.copy(st[:], po[:, t % 2, :])
                else:
                    nc.vector.tensor_copy(out=st[:], in_=po[:, t % 2, :])
                deng = nc.sync if t % 2 == 0 else nc.scalar
                deng.dma_start(out=outb.ap()[t], in_=st)

            for s in range(NG):
                b, k = divmod(s, G_PER_BANK)
                if k == 0:
                    ps = ps1.tile(
                        [P, G_PER_BANK, 8 * TPC], F32, name="ps1", tag="ps1",
                        space="PSUM",
                    )
                nc.tensor.matmul(
                    out=ps[:, k, :],
                    lhsT=wslice(s),
                    rhs=cd[:],
                    start=True,
                    stop=True,
                )
                if k == G_PER_BANK - 1:
                    # scatter-copy the bank into W_sb[ih][:, t, o] (bf16)
                    ih, og0 = b % 2, (b // 2) * 8
                    src = ps[:].rearrange(
                        "p gl (rp t) -> p t gl rp", rp=8, t=TPC
                    )
                    dst = wsb[ih][:][:, :, og0 * 8 : (og0 + 8) * 8].rearrange(
                        "p t (gl rp) -> p t gl rp", gl=G_PER_BANK, rp=8
                    )
                    if b % 2 == 0:
                        nc.vector.tensor_copy(out=dst, in_=src)
                    else:
                        nc.scalar.copy(dst, src)

            # stage 2; drain each type pair as it completes
            for t in range(TPC):
                s2_mm(t)
                if t % 2 == 1:
                    drain(t - 1)
                    drain(t)

    nc.compile()
    return nc


_PROGRAM = None
LAST_RESULT = None  # test harness introspection


def kernel(x, xtype, weight, w_comp, trace=False):
    global _PROGRAM, LAST_RESULT
    x = np.asarray(x, dtype=np.float32)
    xtype = np.asarray(xtype).astype(np.int64)
    weight = np.asarray(weight, dtype=np.float32)
    w_comp = np.asarray(w_comp, dtype=np.float32)
    assert x.shape == (N_FULL, IN_DIM) and weight.shape == (NUM_B, IN_DIM * OUT_DIM)

    if _PROGRAM is None:
        _PROGRAM = _build_program()
    nc = _PROGRAM

    # ---- host-side layout: sort rows by type into padded slots ----
    counts = np.bincount(xtype, minlength=NUM_T)
    if counts.max() > CAP:
        raise RuntimeError(f"type count {counts.max()} exceeds CAP={CAP}")
    order = np.argsort(xtype, kind="stable")
    sorted_t = xtype[order]
    starts = np.zeros(NUM_T, dtype=np.int64)
    starts[1:] = np.cumsum(counts)[:-1]
    rank = np.arange(N_FULL, dtype=np.int64) - starts[sorted_t]
    slot = sorted_t * CAP + rank  # global padded slot per sorted row

    xpad = np.zeros((NUM_T * CAP, IN_DIM), np.float32)
    xpad[slot] = x[order]
    xpad = xpad.astype(NP_BF16)

    # wstack[(r,b), g, j] = weight[b, (ih*128+j)*256 + og*8+r], g = ih*32+og,
    # with columns permuted into the device's step order (banks alternate
    # ih so each o-half of W completes as early as possible).
    w5 = weight.reshape(NUM_B, 2, P, 32, 8)  # b, ih, j, og, r
    wst_nat = np.ascontiguousarray(w5.transpose(4, 0, 1, 3, 2)).reshape(P, NG, P)
    perm = np.empty(NG, np.int64)
    for s in range(NG):
        b, k = divmod(s, G_PER_BANK)
        ih, og = b % 2, (b // 2) * 8 + k
        perm[s] = ih * 32 + og
    wstack = np.ascontiguousarray(wst_nat[:, perm, :]).astype(NP_BF16)

    c_bf = w_comp.astype(NP_BF16)

    in_maps = []
    for c in range(N_CORES):
        xc = xpad[c * TPC * CAP : (c + 1) * TPC * CAP]  # [1024, 256] bf16
        xsT = np.ascontiguousarray(
            xc.reshape(TPC, CAP, 2, P).transpose(3, 2, 0, 1)
        )  # [i, ih, t, n]
        cdl = np.zeros((8, NUM_B, 8, TPC), NP_BF16)  # r, b, rp, t
        for r in range(8):
            cdl[r, :, r, :] = c_bf[c * TPC : (c + 1) * TPC, :].T
        in_maps.append(
            {
                "xsT": xsT,
                "cdelta": cdl.reshape(P, 8 * TPC),
                "wstack": wstack,
            }
        )

    res = run_bass_kernel_spmd(nc, in_maps, list(range(N_CORES)), trace=trace)
    LAST_RESULT = res

    big = np.stack([res.results[c]["outb"] for c in range(N_CORES)]).reshape(
        NUM_T * CAP, OUT_DIM
    )
    out = np.empty((N_FULL, OUT_DIM), np.float32)
    out[order] = big[slot].astype(np.float32)
    return out
